# revision 1
# baseline (speedup 1.0000x reference)
"""Bass/Trainium2 kernel for nn_LSTMModel (B=128, T=512, D=256, H=512).

Sharding: data-parallel over batch across 8 NeuronCores (16 rows each),
weights replicated and SBUF-resident in bf16 transposed (lhsT) layout.

Scan: weight-stationary matmuls, gates on PSUM partitions.
Gate g = mc*128 + p; gate types i = mc 0..3, f = 4..7, g = 8..11, o = 12..15.
h/c state layout [128, (j, b)] with h-index = j*128 + p, so h slices
[:, j*16:(j+1)*16] are directly the K-chunk rhs of the next matmul.

Latency tricks:
- tanh(g) = 2*sigmoid(2g) - 1 with g-weights pre-scaled by 2, so ONE
  sigmoid ACT covers all four gate regions.
- L0 x-side (xp @ wx0.T + b0) block-precomputed per loop iteration with
  wide-N matmuls into SBUF (xg), added to PSUM in one TT op.
- L1 runs one step behind L0 (software pipeline) so each chain's latency
  hides under the other layer's matmul stream.
- L1 uses two PSUM banks ordered [g|i] then [f|o] so its tanh path starts
  after half the matmuls.
"""

import numpy as np

import concourse.bass as bass
import concourse.tile as tile
import concourse.mybir as mybir
from concourse import bacc
from concourse.bass import ds
from concourse.bass_utils import run_bass_kernel_spmd
from concourse.masks import make_identity

F32 = mybir.dt.float32
BF16 = mybir.dt.bfloat16
AF = mybir.ActivationFunctionType
OP = mybir.AluOpType

B, T, D, H = 128, 512, 256, 512
NCORES = 8
BL = B // NCORES            # 16
G = 4 * H                   # 2048
MCH = G // 128              # 16
DKC = D // 128              # 2
HKC = H // 128              # 4

# L1 two-bank split: bank A = [g|i] (finished first -> tanh path starts
# early), bank B = [f|o]. Values are mc indices in emission order.
L1A_MCS = [8, 9, 10, 11, 0, 1, 2, 3]
L1B_MCS = [4, 5, 6, 7, 12, 13, 14, 15]


def build_nc(t_steps=T, unroll=16, mode="real", staggered=True):
    assert t_steps % 8 == 0 and t_steps % unroll == 0
    ntot = t_steps * BL
    nch = 512 if ntot % 512 == 0 else ntot

    nc = bacc.Bacc("TRN2", target_bir_lowering=False)

    x = nc.dram_tensor("x", [BL, t_steps, D], F32, kind="ExternalInput")
    proj_w = nc.dram_tensor("proj_w", [D, D], F32, kind="ExternalInput")
    proj_b = nc.dram_tensor("proj_b", [D], F32, kind="ExternalInput")
    wx0 = nc.dram_tensor("wx0", [G, D], F32, kind="ExternalInput")
    bx0 = nc.dram_tensor("bx0", [G], F32, kind="ExternalInput")
    wh0 = nc.dram_tensor("wh0", [G, H], F32, kind="ExternalInput")
    bh0 = nc.dram_tensor("bh0", [G], F32, kind="ExternalInput")
    wx1 = nc.dram_tensor("wx1", [G, H], F32, kind="ExternalInput")
    bx1 = nc.dram_tensor("bx1", [G], F32, kind="ExternalInput")
    wh1 = nc.dram_tensor("wh1", [G, H], F32, kind="ExternalInput")
    bh1 = nc.dram_tensor("bh1", [G], F32, kind="ExternalInput")
    fc1_w = nc.dram_tensor("fc1_w", [32, H], F32, kind="ExternalInput")
    fc1_b = nc.dram_tensor("fc1_b", [32], F32, kind="ExternalInput")
    fc2_w = nc.dram_tensor("fc2_w", [1, 32], F32, kind="ExternalInput")
    fc2_b = nc.dram_tensor("fc2_b", [1], F32, kind="ExternalInput")
    out_d = nc.dram_tensor("out", [BL, 1], F32, kind="ExternalOutput")

    tens = dict(locals())
    with tile.TileContext(nc) as tc:
        with tc.tile_pool(name="res", bufs=1) as res, \
             tc.tile_pool(name="stg", bufs=3) as stg, \
             tc.tile_pool(name="scn", bufs=3) as scn, \
             tc.tile_pool(name="psum", bufs=2, space="PSUM") as psum:
            _build_body(nc, tc, res, stg, scn, psum, tens, t_steps,
                        unroll, ntot, nch, mode, staggered)
    nc.compile()
    return nc


def _build_body(nc, tc, res, stg, scn, psum, tens, t_steps, unroll, ntot,
                nch, mode, staggered):
    x, out_d = tens["x"], tens["out_d"]
    ublk = unroll * 16

    ident = res.tile([128, 128], F32, tag="ident")
    make_identity(nc, ident[:, :])

    # ---- resident transposed weights (bf16); g-gate rows pre-scaled 2x ----
    w0T = res.tile([128, 6 * G], BF16, tag="w0T")    # kc 0..1 wx0, 2..5 wh0
    w1T = res.tile([128, 8 * G], BF16, tag="w1T")    # kc 0..3 wx1, 4..7 wh1
    for w_d, kcs, dst, kbase in ((tens["wx0"], DKC, w0T, 0),
                                 (tens["wh0"], HKC, w0T, DKC),
                                 (tens["wx1"], HKC, w1T, 0),
                                 (tens["wh1"], HKC, w1T, HKC)):
        cdim = w_d.shape[1]
        for gc in range(MCH):
            st = stg.tile([128, 512], F32, tag="wstage")
            nc.sync.dma_start(out=st[:, 0:cdim],
                              in_=w_d[gc * 128:(gc + 1) * 128, :])
            for kc in range(kcs):
                pt = psum.tile([128, 512], F32, tag="big")
                nc.tensor.transpose(pt[:, 0:128],
                                    st[:, kc * 128:(kc + 1) * 128],
                                    ident[:, :])
                o = ((kbase + kc) * MCH + gc) * 128
                if 8 <= gc <= 11:   # tanh(x) = 2*sigmoid(2x) - 1
                    nc.vector.tensor_scalar_mul(dst[:, o:o + 128],
                                                pt[:, 0:128], 2.0)
                else:
                    nc.vector.tensor_copy(dst[:, o:o + 128], pt[:, 0:128])

    projT = res.tile([128, 2 * D], F32, tag="projT")
    for gc in range(DKC):
        st = stg.tile([128, 512], F32, tag="wstage")
        nc.sync.dma_start(out=st[:, 0:D],
                          in_=tens["proj_w"][gc * 128:(gc + 1) * 128, :])
        for kc in range(DKC):
            pt = psum.tile([128, 512], F32, tag="big")
            nc.tensor.transpose(pt[:, 0:128],
                                st[:, kc * 128:(kc + 1) * 128], ident[:, :])
            nc.vector.tensor_copy(projT[:, (kc * 2 + gc) * 128:
                                        (kc * 2 + gc) * 128 + 128],
                                  pt[:, 0:128])

    fc1T = res.tile([128, HKC * 32], BF16, tag="fc1T")
    st = stg.tile([128, 512], F32, tag="wstage")
    nc.sync.dma_start(out=st[0:32, :], in_=tens["fc1_w"][:, :])
    for kc in range(HKC):
        pt = psum.tile([128, 512], F32, tag="big")
        nc.tensor.transpose(pt[:, 0:32], st[0:32, kc * 128:(kc + 1) * 128],
                            ident[0:32, 0:32])
        nc.vector.tensor_copy(fc1T[:, kc * 32:(kc + 1) * 32], pt[:, 0:32])
    fc2T_f = res.tile([32, 1], F32, tag="fc2T_f")
    nc.sync.dma_start(out=fc2T_f[:, :],
                      in_=tens["fc2_w"][0:1, :].rearrange("o k -> k o"))
    fc2T = res.tile([32, 1], BF16, tag="fc2T")
    nc.vector.tensor_copy(fc2T[:, :], fc2T_f[:, :])
    fc1b = res.tile([32, 1], F32, tag="fc1b")
    nc.sync.dma_start(out=fc1b[:, :],
                      in_=tens["fc1_b"][:].rearrange("(k o) -> k o", o=1))
    fc2b = res.tile([1, 1], F32, tag="fc2b")
    nc.sync.dma_start(out=fc2b[:, :],
                      in_=tens["fc2_b"][:].rearrange("(k o) -> k o", o=1))

    # ---- gate biases: bsum[p, m] = (bx+bh)[m*128+p]; g region scaled 2x ----
    bsums = []
    for ba, bb in ((tens["bx0"], tens["bh0"]), (tens["bx1"], tens["bh1"])):
        parts = []
        for src in (ba, bb):
            st = stg.tile([16, 128], F32, tag="bstage")
            nc.sync.dma_start(out=st[:, :],
                              in_=src[:].rearrange("(m p) -> m p", p=128))
            pt = psum.tile([128, 512], F32, tag="big")
            nc.tensor.transpose(pt[:, 0:16], st[:, :], ident[0:16, 0:16])
            sb = stg.tile([128, 16], F32, tag="btp")
            nc.vector.tensor_copy(sb[:, :], pt[:, 0:16])
            parts.append(sb)
        tot = res.tile([128, 16], F32, tag=f"bsum{len(bsums)}")
        nc.vector.tensor_add(tot[:, :], parts[0][:, :], parts[1][:, :])
        nc.vector.tensor_scalar_mul(tot[:, 8:12], tot[:, 8:12], 2.0)
        bsums.append(tot)

    def bias_cols(bsum, cols_dst, msel):
        for k, m in enumerate(msel):
            nc.vector.tensor_copy(
                cols_dst[:, k * 16:(k + 1) * 16],
                bsum[:, m:m + 1].to_broadcast([128, 16]))

    bias1gi = res.tile([128, 128], F32, tag="bias1gi")
    bias1fo = res.tile([128, 128], F32, tag="bias1fo")
    bias_cols(bsums[1], bias1gi[:, :], L1A_MCS)
    bias_cols(bsums[1], bias1fo[:, :], L1B_MCS)

    # ---- x -> xT (fp32, PE transpose), column order n = t*16 + b ----
    xT = res.tile([128, DKC * ntot], F32, tag="xT")
    for rc in range(t_steps // 8):
        stx = stg.tile([128, 256], F32, tag="xstage")
        for tt in range(8):
            nc.sync.dma_start(
                out=stx[tt * 16:(tt + 1) * 16, :].rearrange(
                    "p (o d) -> p o d", o=1),
                in_=x[:, rc * 8 + tt:rc * 8 + tt + 1, :])
        for kc in range(DKC):
            pt = psum.tile([128, 512], F32, tag="big")
            nc.tensor.transpose(pt[:, 0:128],
                                stx[:, kc * 128:(kc + 1) * 128], ident[:, :])
            nc.vector.tensor_copy(xT[:, kc * ntot + rc * 128:
                                     kc * ntot + rc * 128 + 128],
                                  pt[:, 0:128])

    stp = stg.tile([2, 128], F32, tag="bstage")
    nc.sync.dma_start(out=stp[0:2, :],
                      in_=tens["proj_b"][:].rearrange("(m p) -> m p", p=128))
    ptp = psum.tile([128, 512], F32, tag="big")
    nc.tensor.transpose(ptp[:, 0:2], stp[0:2, :], ident[0:2, 0:2])
    projb_t = res.tile([128, 2], F32, tag="projb")
    nc.vector.tensor_copy(projb_t[:, :], ptp[:, 0:2])

    # ---- xp = x @ proj_w.T + proj_b -> bf16 resident ----
    # padded by one block: the last speculative xg stage reads past T.
    ntot2 = ntot + ublk
    xp = res.tile([128, DKC * ntot2], BF16, tag="xp")
    for kc in range(DKC):
        nc.vector.memset(xp[:, kc * ntot2 + ntot:(kc + 1) * ntot2], 0.0)
    for nt in range(ntot // nch):
        for mc in range(DKC):
            px = psum.tile([128, 512], F32, tag="big")
            for kc in range(DKC):
                nc.tensor.matmul(
                    px[:, 0:nch],
                    projT[:, (kc * 2 + mc) * 128:(kc * 2 + mc) * 128 + 128],
                    xT[:, kc * ntot + nt * nch:kc * ntot + (nt + 1) * nch],
                    start=(kc == 0), stop=(kc == DKC - 1))
            nc.vector.tensor_scalar_add(
                xp[:, mc * ntot2 + nt * nch:mc * ntot2 + (nt + 1) * nch],
                px[:, 0:nch], projb_t[:, mc:mc + 1])

    # ---- scan state ----
    h0_dummy = res.tile([128, 64], BF16, tag="h0d")
    h1_dummy = res.tile([128, 64], BF16, tag="h1d")
    c0 = res.tile([128, 64], F32, tag="c0")
    c1 = res.tile([128, 64], F32, tag="c1")
    h0 = res.tile([128, 64], BF16, tag="h0")
    h1 = res.tile([128, 64], BF16, tag="h1")
    for s_ in (c0, c1, h0, h1, h0_dummy, h1_dummy):
        nc.vector.memset(s_[:, :], 0.0)

    def act(fn, dst, src_):
        nc.scalar.activation(dst, src_, fn)

    def emit_xg_stage(off):
        # fetch one unroll-block of xp columns into a staging tile
        xpb = scn.tile([128, DKC * ublk], BF16, tag="xpb", name="xpb")
        srcv = xp[:, :].rearrange("p (k n) -> p k n", k=DKC)
        nc.sync.dma_start(
            out=xpb[:, :].rearrange("p (k n) -> p k n", k=DKC),
            in_=srcv[:, :, ds(off, ublk)])
        return xpb

    def emit_xg_piece(xpb, xg_sb, mc):
        # xg_sb[:, u*256 + mc*16 + b] = (xp @ wx0.T + b0)[gate chunk mc]
        xgv = xg_sb[:, :].rearrange("p (u r) -> p u r", r=256)
        pt = psum.tile([128, 512], F32, tag="big", name="xgp")
        for kc in range(DKC):
            nc.tensor.matmul(
                pt[:, 0:ublk],
                w0T[:, (kc * MCH + mc) * 128:(kc * MCH + mc) * 128 + 128],
                xpb[:, kc * ublk:(kc + 1) * ublk],
                start=(kc == 0), stop=(kc == DKC - 1))
        nc.vector.tensor_scalar_add(
            xgv[:, :, mc * 16:(mc + 1) * 16],
            pt[:, 0:ublk].rearrange("p (u b) -> p u b", b=16),
            bsums[0][:, mc:mc + 1])

    def emit_l0(ps0):
        # single bank, natural mc order; h-side only (x-side is in xg)
        for mc in range(MCH):
            for j in range(HKC):
                kc = DKC + j
                nc.tensor.matmul(
                    ps0[:, mc * 16:(mc + 1) * 16],
                    w0T[:, (kc * MCH + mc) * 128:(kc * MCH + mc) * 128 + 128],
                    h0[:, j * 16:(j + 1) * 16],
                    start=(j == 0 and mc == 0),
                    stop=(j == HKC - 1 and mc == MCH - 1))

    def emit_l1_part(psl, mcs, src_h, kc_base, start, stop):
        pa, pb = psl
        for s, mc in enumerate(mcs):
            in_a = mc in L1A_MCS
            t_ = pa if in_a else pb
            off = (L1A_MCS.index(mc) if in_a else L1B_MCS.index(mc)) * 16
            for j in range(HKC):
                kc = kc_base + j
                nc.tensor.matmul(
                    t_[:, off:off + 16],
                    w1T[:, (kc * MCH + mc) * 128:(kc * MCH + mc) * 128 + 128],
                    src_h[:, j * 16:(j + 1) * 16],
                    start=(start and j == 0 and s == 0),
                    stop=(stop and j == HKC - 1 and s == len(mcs) - 1))

    def emit_l1_h0side(psl):
        # emitted FIRST: h0(u-1) is already available, so these 64 MMs run
        # stall-free and widen the window that hides chain_l1(u-2) before
        # the h1-side queue head blocks on h1(u-2).
        emit_l1_part(psl, L1A_MCS, h0, 0, True, False)
        emit_l1_part(psl, L1B_MCS, h0, 0, True, False)

    def emit_l1_h1side(psl):
        emit_l1_part(psl, L1A_MCS, h1, HKC, False, True)
        emit_l1_part(psl, L1B_MCS, h1, HKC, False, True)

    def chain_l0(ps0, xg_sb, u):
        # ps0 holds [i|f|2g|o]; one sigmoid covers everything.
        base = u * 256
        hdst = h0 if mode == "real" else h0_dummy
        sig = scn.tile([128, 256], F32, tag="sig0")
        tg = scn.tile([128, 64], F32, tag="tg0")
        tc_ = scn.tile([128, 64], F32, tag="tc0")
        tmp = scn.tile([128, 64], F32, tag="tmp0")
        nc.vector.tensor_add(ps0[:, :], ps0[:, :],
                             xg_sb[:, base:base + 256])
        act(AF.Sigmoid, sig[:, :], ps0[:, :])
        nc.vector.tensor_scalar(tg[:, :], sig[:, 128:192], 2.0, 1.0,
                                OP.mult, OP.subtract)
        nc.vector.tensor_mul(tmp[:, :], sig[:, 0:64], tg[:, :])
        nc.vector.tensor_mul(c0[:, :], sig[:, 64:128], c0[:, :])
        nc.vector.tensor_add(c0[:, :], c0[:, :], tmp[:, :])
        act(AF.Tanh, tc_[:, :], c0[:, :])
        nc.vector.tensor_mul(hdst[:, :], sig[:, 192:256], tc_[:, :])

    def chain_l1(psl):
        # bank A = [g|i], bank B = [f|o]
        pa, pb = psl
        hdst = h1 if mode == "real" else h1_dummy
        sgi = scn.tile([128, 128], F32, tag="sgi1")
        sfo = scn.tile([128, 128], F32, tag="sfo1")
        tg = scn.tile([128, 64], F32, tag="tg1")
        tc_ = scn.tile([128, 64], F32, tag="tc1")
        tmp = scn.tile([128, 64], F32, tag="tmp1")
        nc.vector.tensor_add(pa[:, :], pa[:, :], bias1gi[:, :])
        act(AF.Sigmoid, sgi[:, :], pa[:, :])
        nc.vector.tensor_scalar(tg[:, :], sgi[:, 0:64], 2.0, 1.0,
                                OP.mult, OP.subtract)
        nc.vector.tensor_mul(tmp[:, :], sgi[:, 64:128], tg[:, :])
        nc.vector.tensor_add(pb[:, :], pb[:, :], bias1fo[:, :])
        act(AF.Sigmoid, sfo[:, :], pb[:, :])
        nc.vector.tensor_mul(c1[:, :], sfo[:, 0:64], c1[:, :])
        nc.vector.tensor_add(c1[:, :], c1[:, :], tmp[:, :])
        act(AF.Tanh, tc_[:, :], c1[:, :])
        nc.vector.tensor_mul(hdst[:, :], sfo[:, 64:128], tc_[:, :])

    def l1_tiles():
        return (psum.tile([128, 128], F32, tag="psA", name="psA"),
                psum.tile([128, 128], F32, tag="psB", name="psB"))

    # ---- scan loop: L1 runs one step behind L0; the NEXT block's xg
    # pieces are computed one per step as PE bubble fillers (ping-pong
    # buffers xgA/xgB, written+read only on DVE so ordering is natural).
    assert unroll == MCH
    n_iter = t_steps // unroll
    assert n_iter % 2 == 0
    xgA = res.tile([128, unroll * 256], BF16, tag="xgA")
    xgB = res.tile([128, unroll * 256], BF16, tag="xgB")
    # prologue: xg for block 0
    xpb0 = emit_xg_stage(0)
    for mc in range(MCH):
        emit_xg_piece(xpb0, xgA, mc)

    def subiter(cur_xg, nxt_xg, nxt_off):
        xpb = emit_xg_stage(nxt_off)
        prev_psl = None
        for u in range(unroll):
            ps0 = psum.tile([128, 256], F32, tag="ps0", name="ps0")
            emit_l0(ps0)
            if prev_psl is not None:
                # step u-1's L1: h1-side reads h1(u-2), h0-side reads
                # h0(u-1) -- both must be emitted before chain_l0(u)
                # rewrites h0. chain_l0(u) then hides under these MMs.
                emit_l1_h0side(prev_psl)
                emit_l1_h1side(prev_psl)
            if mode != "nochain":
                chain_l0(ps0, cur_xg, u)
            if prev_psl is not None and mode != "nochain":
                chain_l1(prev_psl)
            prev_psl = l1_tiles()
            # next-block xg piece: independent PE work right where the
            # pipeline stalls on the chains
            emit_xg_piece(xpb, nxt_xg, u)
        # epilogue: L1 of the last step of this subiteration
        emit_l1_h0side(prev_psl)
        emit_l1_h1side(prev_psl)
        if mode != "nochain":
            chain_l1(prev_psl)

    with tc.For_i(0, n_iter // 2, 1,
                  hint_engines=(mybir.EngineType.PE,),
                  staggered_reset=staggered) as it:
        subiter(xgA, xgB, it * (2 * ublk) + ublk)
        subiter(xgB, xgA, it * (2 * ublk) + 2 * ublk)

    # ---- FC head ----
    ph = psum.tile([128, 512], F32, tag="big")
    for kc in range(HKC):
        nc.tensor.matmul(ph[0:32, 0:16], fc1T[:, kc * 32:(kc + 1) * 32],
                         h1[:, kc * 16:(kc + 1) * 16],
                         start=(kc == 0), stop=(kc == HKC - 1))
    hid = scn.tile([32, 16], BF16, tag="hid")
    nc.scalar.activation(hid[:, :], ph[0:32, 0:16], AF.Relu,
                         bias=fc1b[:, 0:1])
    po = psum.tile([128, 512], F32, tag="big")
    nc.tensor.matmul(po[0:1, 0:16], fc2T[:, 0:1], hid[:, :],
                     start=True, stop=True)
    ob = scn.tile([1, 16], F32, tag="ob")
    nc.vector.tensor_scalar_add(ob[:, :], po[0:1, 0:16], fc2b[0:1, 0:1])
    nc.sync.dma_start(out=out_d[:, :].rearrange("b o -> o b"), in_=ob[:, :])


_NC_CACHE = {}


def _get_nc(t_steps=T, unroll=16):
    key = (t_steps, unroll)
    if key not in _NC_CACHE:
        _NC_CACHE[key] = build_nc(t_steps, unroll, "real", staggered=True)
    return _NC_CACHE[key]


def kernel(**inputs):
    nc = _get_nc()
    arrs = {k: np.ascontiguousarray(np.asarray(v, dtype=np.float32))
            for k, v in inputs.items()}
    in_maps = []
    for c in range(NCORES):
        m = {k: v for k, v in arrs.items() if k != "x"}
        m["x"] = np.ascontiguousarray(arrs["x"][c * BL:(c + 1) * BL])
        in_maps.append(m)
    res = run_bass_kernel_spmd(nc, in_maps, core_ids=list(range(NCORES)))
    return np.concatenate([r["out"] for r in res.results], axis=0)

